# revision 11
# baseline (speedup 1.0000x reference)
"""v2: direct-d2sq path (no cancellation bias), fp32 inside-decision chain
with fused custom DVE penalty op. See kernel.py docstring for the math."""

import os
import sys
from contextlib import ExitStack

import numpy as np

for _p in ("/opt/trn_rl_repo",):
    if _p not in sys.path and os.path.isdir(_p):
        sys.path.insert(0, _p)

import concourse.bass as bass
import concourse.tile as tile
from concourse import mybir
from concourse.bass_utils import run_bass_kernel_spmd
from concourse.vector_clock import ScopedClock


# ---------------------------------------------------------------------------
# harness patches (walrus sync-wait limit)
# ---------------------------------------------------------------------------
def _split_drain_and_barrier(self, tick_clock, wait_clock):
    nc = self.nc
    probe = nc.sync.nop(nofuse=True)
    wait_clock.add_sem_waits(probe.ins, ScopedClock({None: tick_clock.global_clock}))
    si = probe.ins.sync_info
    waits = list(si.on_wait or []) if si else []
    if len(waits) > 1:
        probe.ins.sync_info = mybir.SyncInfo(
            on_wait=[waits[0]], on_update=list(si.on_update or [])
        )
        for w in waits[1:]:
            nop = nc.sync.nop(nofuse=True)
            nop.ins.sync_info = mybir.SyncInfo(on_wait=[w], on_update=[])
    nc.sync.drain()
    nc.all_engine_barrier()
    popped = nc._tile_sem_poison_stack.pop()
    assert popped is self._sem_poison
    # clear sems for repeat executions via per-sem writes: the
    # EVENT_SEMAPHORE_RANGE_CLEAR ISA op of clear_and_free_semaphores does
    # not encode on this walrus build ("ISA wrong length").
    for sem in list(self.sems.allocated().values()):
        num = sem.num if hasattr(sem, "num") else int(sem)
        nm = str(getattr(sem, "name", "") or f"sem{num}")
        nop = nc.sync.nop(nofuse=True)
        nop.ins.sync_info = mybir.SyncInfo(
            on_wait=[],
            on_update=[
                mybir.SyncUpdate(
                    sync_type="semaphore",
                    id=num,
                    update_mode="sem-wr-imm",
                    update_value=0,
                    ant_name=nm,
                )
            ],
        )
    nc.all_engine_barrier()


tile.TileContext._drain_and_barrier = _split_drain_and_barrier

_DMA_INST_TYPES = ("InstTensorLoad", "InstTensorSave", "InstTensorCopy")


def _reduce_waits(nc):
    """Transitive reduction of sem waits (Tile's pass 1B is not transitively
    minimal per-proc chain): drop wait W on instruction I when I's OTHER waits
    already imply W, or when a wait observed by an earlier same-stream
    instruction already implies it. Engines are pipelined, so a stream
    predecessor's own COMPLETION is never assumed — only its observed waits
    (issue-ordered) and explicit sem implications (completion-ordered) are
    used. Needed because this walrus build allows only one sync wait on DMA
    descriptors."""
    tainted = set()
    sem_hist = {}  # sem -> list[(cum_after, complete_vc_of_updater)]
    sem_cum = {}
    issue_vc = {}  # stream -> VC of everything observed via waits at issue
    complete_vc = {}  # stream -> VC implied by this stream's completions
    # All nc.sync HWDGE DMAs share one physical ring (qSPDynamicHW) and each
    # SDMA engine drains it FIFO, with a fixed partition->engine map - so two
    # SP DMAs are ordered on any overlapping SBUF region regardless of which
    # DMAHW bookkeeping lane Tile assigned them. A wait on a DMAHW lane value
    # already reached by earlier SP-issued DMAs is therefore redundant.
    sp_dma_cum = {}

    def stream_of(inst):
        if type(inst).__name__ == "InstDMACopy":
            si = inst.sync_info
            if si and si.on_update:
                return ("S", si.on_update[0].ant_name)
            return ("I", inst.name)
        return ("E", str(inst.engine))

    def vc_join(dst, src):
        for k, v in src.items():
            if dst.get(k, 0) < v:
                dst[k] = v

    def implied_vc(w):
        if w.wait_mode != "sem-ge-imm" or w.ant_name in tainted:
            return None
        for cum, vc in sem_hist.get(w.ant_name, []):
            if cum >= w.wait_value:
                return vc
        return None

    trace = []
    for blk in nc.m.functions[0].blocks:
        for inst in blk.instructions:
            si = inst.sync_info
            waits = list(si.on_wait) if si and si.on_wait else []
            orig_n = len(waits)
            upds = list(si.on_update) if si and si.on_update else []
            s = stream_of(inst)
            is_sp_dma = type(inst).__name__ == "InstDMACopy" and str(
                inst.engine
            ).endswith("SP")
            if is_sp_dma:
                waits = [
                    w
                    for w in waits
                    if not (
                        w.wait_mode == "sem-ge-imm"
                        and w.ant_name.startswith("DMAHW")
                        and sp_dma_cum.get(w.ant_name, 0) >= w.wait_value
                    )
                ]
            covered = dict(issue_vc.get(s, {}))
            cum_before_rec = None
            infos = []
            for w in waits:
                iv = implied_vc(w)
                droppable = (
                    w.wait_mode == "sem-ge-imm"
                    and w.ant_name not in tainted
                    and w.ant_name in sem_cum
                )
                infos.append((w, iv, droppable, sum((iv or {}).values())))
            infos.sort(key=lambda t: -t[3])
            keep = []
            for w, iv, droppable, _ in infos:
                if droppable and covered.get(w.ant_name, 0) >= w.wait_value:
                    continue
                keep.append(w)
                if iv is not None:
                    vc_join(covered, iv)
                if w.wait_mode == "sem-ge-imm":
                    if covered.get(w.ant_name, 0) < w.wait_value:
                        covered[w.ant_name] = w.wait_value
            issue_vc[s] = covered
            comp = dict(complete_vc.get(s, {}))
            vc_join(comp, covered)
            for u in upds:
                if cum_before_rec is None:
                    cum_before_rec = sem_cum.get(u.ant_name, 0)
                if u.update_mode not in ("sem-add-imm", "sem-inc"):
                    tainted.add(u.ant_name)
                    continue
                inc = u.update_value if u.update_mode == "sem-add-imm" else (
                    u.update_value or 1
                )
                sem_cum[u.ant_name] = sem_cum.get(u.ant_name, 0) + inc
                comp[u.ant_name] = sem_cum[u.ant_name]
                sem_hist.setdefault(u.ant_name, []).append(
                    (sem_cum[u.ant_name], dict(comp))
                )
                if is_sp_dma:
                    sp_dma_cum[u.ant_name] = sem_cum[u.ant_name]
            complete_vc[s] = comp
            if len(keep) != orig_n:
                order = {id(w): k for k, w in enumerate(waits)}
                keep.sort(key=lambda w: order[id(w)])
                inst.sync_info = mybir.SyncInfo(on_wait=keep, on_update=upds)
            trace.append((inst, s, cum_before_rec, type(inst).__name__))
    return trace, sem_hist


def _distribute_waits(nc, trace, sem_hist, limit=1):
    """Pack sem waits so no instruction carries more than `limit` (walrus
    rejects >1 sync wait per instruction). Per stream (engine / DMA queue),
    each movable wait W may sit on any same-stream instruction between its
    earliest-safe position (the first spot whose preceding stream-sem count
    covers everything W's producer transitively needs from this stream - so
    stalling there cannot starve the producer) and its original instruction.
    Right-to-left greedy: at each slot place the most-constrained available
    wait. Unplaceable waits stay put (and we warn)."""
    by_stream = {}
    for rec in trace:
        by_stream.setdefault(rec[1], []).append(rec)

    def hist_vc(sem, val):
        for cum, vc in sem_hist.get(sem, []):
            if cum >= val:
                return vc
        return None

    stream_sem = {}
    for inst, s, _, _ in trace:
        si = inst.sync_info
        if si and si.on_update and s[0] == "E":
            nm = si.on_update[0].ant_name
            if not nm.startswith("barrier"):
                stream_sem.setdefault(s, nm)

    holder_types = (
        "InstTensorTensor",
        "InstTensorScalarPtr",
        "InstActivation",
        "InstCustomDveAnt",
        "InstReciprocal",
        "InstMemset",
        "InstNoOp",
        "InstDrain",
    )

    for s, recs in by_stream.items():
        if s[0] != "E":
            continue
        esem = stream_sem.get(s)
        if esem is None:
            continue
        n = len(recs)
        fixed = [[] for _ in range(n)]  # immovable waits per slot
        movable = []  # (esp_idx, orig_idx, wait)
        cums = [rec[2] for rec in recs]  # cum_before per slot (may be None)

        def first_idx_with_cum(need, hi):
            # first slot index whose cum_before >= need (search 0..hi)
            for k in range(hi + 1):
                if cums[k] is not None and cums[k] >= need:
                    return k
            return hi

        any_over = False
        for i, (inst, _, _, tname) in enumerate(recs):
            si = inst.sync_info
            waits = list(si.on_wait) if si and si.on_wait else []
            # dedup same-sem ge waits
            best, rest = {}, []
            for w in waits:
                if w.wait_mode == "sem-ge-imm":
                    k = w.ant_name
                    if k not in best or best[k].wait_value < w.wait_value:
                        best[k] = w
                else:
                    rest.append(w)
            waits = list(best.values()) + rest
            if len(waits) > limit:
                any_over = True
            for w in waits:
                mv = (
                    w.wait_mode == "sem-ge-imm"
                    and not w.ant_name.startswith("barrier")
                )
                if mv:
                    if w.ant_name == esem:
                        need = w.wait_value
                        vcok = True
                    else:
                        vc = hist_vc(w.ant_name, w.wait_value)
                        vcok = vc is not None
                        need = vc.get(esem, 0) if vc is not None else 0
                    if vcok:
                        movable.append((first_idx_with_cum(need, i), i, w))
                        continue
                fixed[i].append(w)
        if not any_over:
            continue

        cap = []
        for i, (inst, _, _, tname) in enumerate(recs):
            c = (1 if tname in holder_types + ("InstActivation",) else 0) - len(
                fixed[i]
            )
            cap.append(max(c, 0) if tname in holder_types else 0)

        # right-to-left greedy: most-constrained (largest esp) first
        placed = {}  # slot -> list of waits
        pending = sorted(movable, key=lambda t: t[1])  # by orig idx
        unassigned = list(pending)
        for idx in range(n - 1, -1, -1):
            while cap[idx] > 0:
                candidates = [
                    t for t in unassigned if t[0] <= idx <= t[1]
                ]
                if not candidates:
                    break
                t = max(candidates, key=lambda t: t[0])
                unassigned.remove(t)
                placed.setdefault(idx, []).append(t[2])
                cap[idx] -= 1
        for t in unassigned:
            placed.setdefault(t[1], []).append(t[2])  # leave at origin

        for i, (inst, _, _, tname) in enumerate(recs):
            si = inst.sync_info
            new_waits = fixed[i] + placed.get(i, [])
            old = list(si.on_wait) if si and si.on_wait else []
            if len(new_waits) > limit:
                sys.stderr.write(
                    f"WARN: {inst.name} still has {len(new_waits)} waits\n"
                )
            if [id(w) for w in old] != [id(w) for w in new_waits]:
                inst.sync_info = mybir.SyncInfo(
                    on_wait=new_waits,
                    on_update=list(si.on_update or []) if si else [],
                )



# ---------------------------------------------------------------------------
N_CORES = 8
B, H, W = 4, 512, 512
HPC = H // N_CORES
P = 128
FTOT = B * HPC * W // P
F = 128
NCH = FTOT // F
NI, NJ, NC5 = 9, 4, 5
BIG = 60000.0

f32 = mybir.dt.float32
f16 = mybir.dt.float16
ALU = mybir.AluOpType
ACTF = mybir.ActivationFunctionType


def _emit_chunk(nc, tc, pools, dram, c):
    st, small, istk, mega, half, acc = pools
    xoff, yoff, flow, out = dram
    V = nc.vector
    S = nc.scalar

    # ---- load fp32 input slabs ----
    xs = st.tile([P, NI, F], f32, name="xs", tag="xs")
    nc.sync.dma_start(out=xs[:], in_=xoff[:, c])
    ys = st.tile([P, NI, F], f32, name="ys", tag="ys")
    nc.sync.dma_start(out=ys[:], in_=yoff[:, c])
    fs = st.tile([P, NC5, F], f32, name="fs", tag="fs")
    nc.sync.dma_start(out=fs[:], in_=flow[:, c])

    # ---- fp16 working set (doubled) ----
    X2 = istk.tile([P, NI, F], f16, name="X2", tag="X2")
    S.mul(X2[:], xs[:], 2.0)
    Y2 = istk.tile([P, NI, F], f16, name="Y2", tag="Y2")
    S.mul(Y2[:], ys[:], 2.0)
    FL = small.tile([P, NC5, F], f16, name="FL", tag="FL")
    S.copy(FL[:], fs[:])
    FL2 = small.tile([P, NC5, F], f16, name="FL2", tag="FL2")
    V.tensor_scalar(FL2[:], FL[:], 2.0, None, ALU.mult)

    # ---- hoisted per-j planes (decision chain in fp32) ----
    squ32 = small.tile([P, NC5, F], f32, name="squ32", tag="squ32")
    S.square(squ32[:], FL[:])
    uuvv32 = small.tile([P, NJ, F], f32, name="uuvv32", tag="uuvv32")
    V.tensor_tensor(uuvv32[:], squ32[:, 0:4], squ32[:, 1:5], ALU.add)
    uuvv2 = small.tile([P, NJ, F], f32, name="uuvv2", tag="uuvv2", bufs=1)
    V.tensor_scalar(uuvv2[:], uuvv32[:], 2.0, None, ALU.mult)
    uuvvc = small.tile([P, NJ, F], f32, name="uuvvc", tag="uuvvc", bufs=1)
    V.tensor_scalar(uuvvc[:], uuvv32[:], 1e-12, None, ALU.max)
    inv32 = small.tile([P, NJ, F], f32, name="inv32", tag="inv32", bufs=1)
    V.reciprocal(inv32[:], uuvvc[:])
    inv4 = small.tile([P, NJ, F], f16, name="inv4", tag="inv4")
    V.tensor_scalar(inv4[:], inv32[:], 0.25, BIG, ALU.mult, ALU.min)

    x2d = istk.tile([P, NI, F], f32, name="x2d", tag="x2d", bufs=1)
    S.activation(x2d[:], X2[:], ACTF.Square, scale=float(np.sqrt(0.5)))
    sqx = istk.tile([P, NI, F], f16, name="sqx", tag="iw", bufs=6)
    S.activation(sqx[:], X2[:], ACTF.Square, scale=0.5)
    sqy = istk.tile([P, NI, F], f16, name="sqy", tag="iw", bufs=6)
    S.activation(sqy[:], Y2[:], ACTF.Square, scale=0.5)
    d1sq = istk.tile([P, NI, F], f16, name="d1sq", tag="d1sq")
    V.tensor_tensor(d1sq[:], sqx[:], sqy[:], ALU.add)

    # ---- shared products ----
    GX = mega.tile([P, NC5, NI, F], f16, name="GX", tag="GX", bufs=2)
    V.tensor_tensor(
        GX[:],
        X2.unsqueeze(1).to_broadcast((P, NC5, NI, F)),
        FL.unsqueeze(2).to_broadcast((P, NC5, NI, F)),
        ALU.mult,
    )
    GY = mega.tile([P, NC5, NI, F], f16, name="GY", tag="GY", bufs=2)
    V.tensor_tensor(
        GY[:],
        Y2.unsqueeze(1).to_broadcast((P, NC5, NI, F)),
        FL.unsqueeze(2).to_broadcast((P, NC5, NI, F)),
        ALU.mult,
    )

    inv4_bc = inv4.unsqueeze(2).to_broadcast((P, NJ, NI, F))
    x2d_bc = x2d.unsqueeze(1).to_broadcast((P, NJ, NI, F))
    uuvv2_bc = uuvv2.unsqueeze(2).to_broadcast((P, NJ, NI, F))

    # ---- inside-test penalty (fp32 decision, fused) ----
    t2 = mega.tile([P, NJ, NI, F], f32, name="t2", tag="t2", bufs=1)
    V.tensor_tensor(t2[:], x2d_bc, GY[:, 1:5], ALU.add)
    # fp16 outputs preserve the SIGN of the fp32-computed values exactly,
    # and only sign(g) = sign(t2)*sign(t2-2uuvv) feeds the decision.
    tsub = mega.tile([P, NJ, NI, F], f16, name="tsub", tag="mg")
    V.tensor_tensor(tsub[:], t2[:], uuvv2_bc, ALU.subtract)
    g = mega.tile([P, NJ, NI, F], f16, name="g", tag="mg")
    V.tensor_tensor(g[:], t2[:], tsub[:], ALU.mult)
    pen = mega.tile([P, NJ, NI, F], f16, name="pen", tag="mg")
    V.tensor_scalar(pen[:], g[:], 0.0, BIG, ALU.is_gt, ALU.mult)

    # ---- perp^2 candidate ----
    pnum2 = mega.tile([P, NJ, NI, F], f16, name="pnum2", tag="mg")
    V.tensor_tensor(pnum2[:], GX[:, 1:5], GY[:, 0:4], ALU.subtract)
    p2 = mega.tile([P, NJ, NI, F], f16, name="p2", tag="mg")
    S.square(p2[:], pnum2[:])
    perp2 = mega.tile([P, NJ, NI, F], f16, name="perp2", tag="mg")
    V.tensor_tensor(perp2[:], p2[:], inv4_bc, ALU.mult)
    cand = mega.tile([P, NJ, NI, F], f16, name="cand", tag="mg")
    V.tensor_tensor(cand[:], perp2[:], pen[:], ALU.add)

    # ---- endpoint distances d2sq (direct, no cancellation) ----
    DXU = mega.tile([P, NJ, NI, F], f16, name="DXU", tag="mg")
    V.tensor_tensor(
        DXU[:],
        X2.unsqueeze(1).to_broadcast((P, NJ, NI, F)),
        FL2[:, 0:4].unsqueeze(2).to_broadcast((P, NJ, NI, F)),
        ALU.subtract,
    )
    DYV = mega.tile([P, NJ, NI, F], f16, name="DYV", tag="mg")
    V.tensor_tensor(
        DYV[:],
        Y2.unsqueeze(1).to_broadcast((P, NJ, NI, F)),
        FL2[:, 1:5].unsqueeze(2).to_broadcast((P, NJ, NI, F)),
        ALU.subtract,
    )
    A2 = mega.tile([P, NJ, NI, F], f16, name="A2", tag="mg")
    S.activation(A2[:], DXU[:], ACTF.Square, scale=0.5)
    B2 = mega.tile([P, NJ, NI, F], f16, name="B2", tag="mg")
    S.activation(B2[:], DYV[:], ACTF.Square, scale=0.5)
    d2sq = mega.tile([P, NJ, NI, F], f16, name="d2sq", tag="mg")
    V.tensor_tensor(d2sq[:], A2[:], B2[:], ALU.add)

    # ---- min over j ----
    mC1 = half.tile([P, 2, NI, F], f16, name="mC1", tag="half")
    V.tensor_tensor(mC1[:], cand[:, 0:2], cand[:, 2:4], ALU.min)
    mC = istk.tile([P, NI, F], f16, name="mC", tag="iw", bufs=6)
    V.tensor_tensor(mC[:], mC1[:, 0], mC1[:, 1], ALU.min)
    mD1 = half.tile([P, 2, NI, F], f16, name="mD1", tag="half")
    V.tensor_tensor(mD1[:], d2sq[:, 0:2], d2sq[:, 2:4], ALU.min)
    mD = istk.tile([P, NI, F], f16, name="mD", tag="iw", bufs=6)
    V.tensor_tensor(mD[:], mD1[:, 0], mD1[:, 1], ALU.min)

    # ---- m = min(mC, mD, d1sq)  (all >= 0 by construction) ----
    m1 = istk.tile([P, NI, F], f16, name="m1", tag="iw", bufs=6)
    V.tensor_tensor(m1[:], mC[:], mD[:], ALU.min)
    m = istk.tile([P, NI, F], f16, name="m", tag="iw", bufs=6)
    V.tensor_tensor(m[:], m1[:], d1sq[:], ALU.min)

    # ---- sqrt + fused free-dim sum ----
    sq = istk.tile([P, NI, F], f16, name="sq", tag="iw", bufs=6)
    part = acc.tile([P, 1], f32, name="part", tag="part")
    S.activation(sq[:], m[:], ACTF.Sqrt, accum_out=part[:])
    nc.sync.dma_start(out=out[:, c : c + 1], in_=part[:])


def build_program():
    nc = bass.Bass()
    xoff = nc.declare_dram_parameter("xoff", [P, NCH, NI, F], f32, isOutput=False)
    yoff = nc.declare_dram_parameter("yoff", [P, NCH, NI, F], f32, isOutput=False)
    flow = nc.declare_dram_parameter("flow", [P, NCH, NC5, F], f32, isOutput=False)
    out = nc.declare_dram_parameter("out", [P, NCH], f32, isOutput=True)

    with tile.TileContext(nc) as tc:
        with ExitStack() as ctx:
            st = ctx.enter_context(tc.tile_pool(name="stage", bufs=2))
            small = ctx.enter_context(tc.tile_pool(name="small", bufs=2))
            istk = ctx.enter_context(tc.tile_pool(name="istk", bufs=2))
            mega = ctx.enter_context(tc.tile_pool(name="mega", bufs=4))
            half = ctx.enter_context(tc.tile_pool(name="half", bufs=2))
            acc = ctx.enter_context(tc.tile_pool(name="acc", bufs=8))
            pools = (st, small, istk, mega, half, acc)
            dram = (xoff, yoff, flow, out)
            for c in range(NCH):
                _emit_chunk(nc, tc, pools, dram, c)
    trace, sem_hist = _reduce_waits(nc)
    _distribute_waits(nc, trace, sem_hist, limit=1)
    return nc


def _host_layout(arr_c, nplanes):
    a = arr_c.reshape(B, nplanes, HPC // 2, 2, W // F, F)
    a = a.transpose(0, 2, 3, 4, 1, 5)
    return np.ascontiguousarray(a.reshape(P, NCH, nplanes, F))


_PROGRAM = None


def kernel(offset: np.ndarray, optical_flow: np.ndarray) -> np.ndarray:
    global _PROGRAM
    offset = np.asarray(offset, dtype=np.float32)
    optical_flow = np.asarray(optical_flow, dtype=np.float32)
    assert offset.shape == (B, 18, H, W) and optical_flow.shape == (B, 8, H, W)

    if _PROGRAM is None:
        _PROGRAM = build_program()
    nc = _PROGRAM

    in_maps = []
    for k in range(N_CORES):
        hs = k * HPC
        sl = slice(hs, hs + HPC)
        in_maps.append(
            {
                "xoff": _host_layout(offset[:, 0:9, sl], NI),
                "yoff": _host_layout(offset[:, 9:18, sl], NI),
                "flow": _host_layout(optical_flow[:, 0:5, sl], NC5),
            }
        )

    res = run_bass_kernel_spmd(nc, in_maps, core_ids=list(range(N_CORES)))
    total = np.float64(0.0)
    for r in res.results:
        total += r["out"].astype(np.float64).sum()
    return np.float32(total / (NI * H * W))
